# revision 23
# baseline (speedup 1.0000x reference)
"""ArcFace head on 8 TRN2 NeuronCores (Bass/Tile).

Model-parallel over classes: each of the 8 cores owns a 12500-class slice
of the 100000-class weight matrix and computes its (1024 x 12500) slice of
the logits; the host reassembles slices along the class dim.

v8: the device does ONLY the GEMM + psum->bf16 drains + output DMA.
Both operands are L2-normalized on the host (the x64 logit scale is
folded into the weights), so each drain is a single dtype-cast copy.
The ArcFace margin touches one column per row; the host computes those
1024 corrected logits exactly and overlays them during reassembly.

Schedule: window 0 runs k-major across all 8 psum banks so matmuls can
start as soon as the first (embT k0, w0 k0) pieces land; those gating
pieces ride the two hardware DGE queues (sync/scalar — the gpsimd queue
is a software DGE with ~5-6us per-transfer latency and only carries
work whose latency is hidden). Later windows run bt-major with a
4-window weight prefetch and per-2bt output DMAs issued right after
each drain pair, round-robin across the 3 queues; the last 3 windows
go per-bt on the hardware queues only so the teardown barrier never
waits on a late software-DGE transfer. Drains alternate Act/DVE.
Dummy matmuls warm the PE clock gate during the input fill, sized to
end right at data-ready so the real stream runs at 2.4 GHz throughout.
Measured ~189.5us on 8 cores (800 matmuls x 211ns = 169.6us PE floor).
"""

import math

import ml_dtypes
import numpy as np

import concourse.bacc as bacc
import concourse.bass as bass  # noqa: F401  (kept for parity with tooling)
import concourse.mybir as mybir
import concourse.tile as tile

# Problem constants (hardcoded per harness rules).
B = 1024  # batch
D = 512  # embedding dim
C = 100000  # num classes
NCORES = 8
CS = C // NCORES  # classes per core = 12500
P = 128  # partitions
KCH = D // P  # contraction chunks = 4
NB = B // P  # batch tiles = 8
CW = 500  # class window (<=512 psum bank, divides 12500)
NCW = CS // CW  # 25 class windows

SCALE = 64.0
MARGIN = 0.5
COS_M = math.cos(MARGIN)
SIN_M = math.sin(MARGIN)
TH = math.cos(math.pi - MARGIN)
MM = math.sin(math.pi - MARGIN) * MARGIN

F32 = mybir.dt.float32
BF16 = mybir.dt.bfloat16


def build_graph():
    nc = bacc.Bacc(
        "TRN2",
        target_bir_lowering=False,
        debug=False,
        num_devices=NCORES,
    )

    embT_l = nc.declare_dram_parameter("embT_l", [P, KCH, B], BF16, isOutput=False)
    wt_l = nc.declare_dram_parameter("wt_l", [NCW, P, KCH, CW], BF16, isOutput=False)
    out_dev = nc.declare_dram_parameter(
        "out_dev", [NCW, P, NB, CW], BF16, isOutput=True
    )

    with tile.TileContext(nc) as tc:
        with (
            tc.tile_pool(name="const", bufs=1) as constp,
            tc.tile_pool(name="embp", bufs=1) as embp,
            tc.tile_pool(name="wstage", bufs=6) as wstage,
            tc.tile_pool(name="ostripe", bufs=3) as ostripep,
            tc.tile_pool(name="ps_main", bufs=8, space="PSUM") as ps_main,
        ):
            # ---------- input staging FIRST on the 3 HWDGE queues (sync,
            # scalar, gpsimd) so no preamble op delays a DMA issue: embT
            # half-k pieces on sync+scalar, w0 k-quarters on gpsimd,
            # matching the k-major consumption order of window 0.
            # embT and w0 are PER-K tiles: the tile tracker orders readers
            # after every writer of a tile, so a single staging tile would
            # stall the first matmul until the last k piece lands.
            embT_k = [
                embp.tile([P, B], BF16, tag=f"embT{k}", name=f"embT{k}")
                for k in range(KCH)
            ]
            wt_tiles = {}
            wt0_k = [
                embp.tile([P, CW], BF16, tag=f"wt0_{k}", name=f"wt0_{k}")
                for k in range(KCH)
            ]

            # staging on the two hardware DGE queues (sync, scalar) only —
            # the gpsimd queue is a software DGE with ~5-6us per-transfer
            # latency. The gating pieces for the k-major window 0 come
            # first: embT k0 halves, then w0-k0 halves, then the rest in
            # consumption order. 5 staging DMAs per queue (more stalls the
            # issue credits).
            H = B // 2
            HW = CW // 2
            nc.sync.dma_start(out=embT_k[0][:, :H], in_=embT_l[:, 0, :H])
            nc.scalar.dma_start(out=embT_k[0][:, H:], in_=embT_l[:, 0, H:])
            nc.sync.dma_start(out=wt0_k[0][:, :HW], in_=wt_l[0, :, 0, :HW])
            nc.scalar.dma_start(out=wt0_k[0][:, HW:], in_=wt_l[0, :, 0, HW:])
            for k in range(1, KCH):
                nc.sync.dma_start(out=embT_k[k][:, :H], in_=embT_l[:, k, :H])
                nc.scalar.dma_start(out=embT_k[k][:, H:], in_=embT_l[:, k, H:])
            # w0 k1..k3 on gpsimd: their ~6us SWDGE latency is hidden
            # behind the k0 round of window 0
            for k in range(1, KCH):
                nc.gpsimd.dma_start(out=wt0_k[k][:], in_=wt_l[0, :, k, :])

            QS = [nc.sync, nc.scalar, nc.gpsimd]

            def wt_dma(cw):
                wt_f = wstage.tile([P, KCH, CW], BF16, tag="wt_f")
                QS[cw % 3].dma_start(out=wt_f[:], in_=wt_l[cw])
                wt_tiles[cw] = wt_f

            # early prefetch: w1 split across sync+scalar so it lands well
            # before window 1 starts; w2..w4 whole-window round-robin
            wt1 = wstage.tile([P, KCH, CW], BF16, tag="wt_f")
            nc.sync.dma_start(out=wt1[:, :2, :], in_=wt_l[1, :, :2, :])
            nc.scalar.dma_start(out=wt1[:, 2:, :], in_=wt_l[1, :, 2:, :])
            wt_tiles[1] = wt1
            wt_dma(2)
            wt_dma(3)
            wt_dma(4)

            # preamble compute (after every staging DMA is in flight):
            # Act Copy table preload so no ACT_TABLE_LOAD lands mid-pipe,
            # then HAM warmup matmuls on memset data, sized to end right at
            # data-ready (~11.5us) so the real stream starts at 2.4 GHz.
            actwarm = constp.tile([1, 8], F32, tag="actwarm")
            nc.scalar.copy(actwarm[:], actwarm[:])
            warm_rhs = constp.tile([P, 512], BF16, tag="warm_rhs")
            nc.vector.memset(warm_rhs[:], 0.0)
            warm_ps = ps_main.tile([P, 512], F32, tag="ps_main")
            for _ in range(11):
                nc.tensor.matmul(
                    warm_ps[:1, :], lhsT=warm_rhs[:, :1], rhs=warm_rhs[:],
                    start=True, stop=True,
                )

            def embT_lhs(bt, k):
                o = bt * P
                return embT_k[k][:, o : o + P]

            def drain(ostripe, po, bt):
                # single dtype-cast copy psum f32 -> bf16, alternating engines
                if bt % 2 == 0:
                    nc.scalar.copy(ostripe[:, bt, :], po[:, :CW])
                else:
                    nc.vector.tensor_copy(ostripe[:, bt, :], po[:, :CW])

            # ---------- window 0: k-major over all 8 psum banks
            ostripe0 = ostripep.tile([P, NB, CW], BF16, tag="ostripe")
            po0 = [
                ps_main.tile([P, 512], F32, tag="ps_main", name=f"po0_{i}")
                for i in range(NB)
            ]
            for k in range(KCH):
                for bt in range(NB):
                    nc.tensor.matmul(
                        po0[bt][:, :CW],
                        lhsT=embT_lhs(bt, k),
                        rhs=wt0_k[k][:],
                        start=(k == 0),
                        stop=(k == KCH - 1),
                    )
            for bt in range(NB):
                drain(ostripe0, po0[bt], bt)
                if bt % 2 == 1:
                    QS[(bt // 2) % 3].dma_start(
                        out=out_dev[0, :, bt - 1 : bt + 1, :],
                        in_=ostripe0[:, bt - 1 : bt + 1, :],
                    )

            # ---------- windows 1..24: bt-major
            for cw in range(1, NCW):
                if cw + 4 < NCW:
                    wt_dma(cw + 4)
                ostripe = ostripep.tile([P, NB, CW], BF16, tag="ostripe")
                for bt in range(NB):
                    po = ps_main.tile([P, 512], F32, tag="ps_main")
                    for k in range(KCH):
                        nc.tensor.matmul(
                            po[:, :CW],
                            lhsT=embT_lhs(bt, k),
                            rhs=wt_tiles[cw][:, k, :],
                            start=(k == 0),
                            stop=(k == KCH - 1),
                        )
                    drain(ostripe, po, bt)
                    # output DMAs issue right after each drain (pair) so no
                    # queue ever backlogs more than ~one chunk: per-2bt
                    # chunks normally; the last 3 windows go per-bt on the
                    # two HARDWARE queues only (the gpsimd software DGE has
                    # ~5us per-transfer latency, which would land after the
                    # stream ends and stall the teardown barrier).
                    if cw >= NCW - 3:
                        QS[bt % 2].dma_start(
                            out=out_dev[cw, :, bt : bt + 1, :],
                            in_=ostripe[:, bt : bt + 1, :],
                        )
                    elif bt % 2 == 1:
                        QS[(cw + bt // 2) % 3].dma_start(
                            out=out_dev[cw, :, bt - 1 : bt + 1, :],
                            in_=ostripe[:, bt - 1 : bt + 1, :],
                        )

    nc.compile()
    return nc


def make_in_maps(embeddings, labels, weight):
    """Host-side layout prep: L2-normalize, fold the x64 scale into the
    weights, shard the weights over classes, transpose for the PE."""
    emb = np.asarray(embeddings, dtype=np.float32)
    w = np.asarray(weight, dtype=np.float32)

    bf16 = ml_dtypes.bfloat16

    en = emb / np.maximum(
        np.sqrt((emb * emb).sum(axis=1, keepdims=True)), 1e-12
    )
    wn = w / np.maximum(np.sqrt((w * w).sum(axis=1, keepdims=True)), 1e-12)
    wn *= SCALE

    # embT_l[p, k, b] = en[b, k*128+p]
    embT_l = np.ascontiguousarray(
        en.T.reshape(KCH, P, B).transpose(1, 0, 2)
    ).astype(bf16)

    in_maps = []
    for c in range(NCORES):
        wsh = wn[c * CS : (c + 1) * CS]
        # wt_l[cw, p, k, cl] = wsh[cw*500+cl, k*128+p]
        wt_l = np.ascontiguousarray(
            wsh.T.reshape(KCH, P, NCW, CW).transpose(2, 1, 0, 3)
        ).astype(bf16)
        in_maps.append({"embT_l": embT_l, "wt_l": wt_l})
    return in_maps


def assemble_output(results, embeddings, labels, weight):
    """Host-side reassembly: window-major device blocks -> (B, C) f32,
    then overlay the exact margin-corrected target logits."""
    lab = np.asarray(labels).astype(np.int64)
    emb = np.asarray(embeddings, dtype=np.float32)
    w = np.asarray(weight, dtype=np.float32)

    out = np.empty((B, C), dtype=np.float32)
    for c in range(NCORES):
        blk = np.asarray(results[c]["out_dev"]).astype(np.float32)
        # blk[cw, p, i, cl] -> out[i*128+p, c*CS + cw*500 + cl]
        out[:, c * CS : (c + 1) * CS] = blk.transpose(2, 1, 0, 3).reshape(B, CS)

    # exact target-column margin, computed like the reference (f32)
    en = emb / np.maximum(
        np.sqrt((emb * emb).sum(axis=1, keepdims=True)), 1e-12
    )
    wl = w[lab]
    wln = wl / np.maximum(
        np.sqrt((wl * wl).sum(axis=1, keepdims=True)), 1e-12
    )
    cos = np.clip((en * wln).sum(axis=1), -1.0 + 1e-7, 1.0 - 1e-7)
    sin = np.sqrt(1.0 - cos * cos)
    cosm = cos * COS_M - sin * SIN_M
    tgt = np.where(cos > TH, cosm, cos - MM) * SCALE
    out[np.arange(B), lab] = tgt.astype(np.float32)
    return out


_CACHED_NC = None


def _get_graph():
    global _CACHED_NC
    if _CACHED_NC is None:
        _CACHED_NC = build_graph()
    return _CACHED_NC


def kernel(embeddings, labels, weight):
    from concourse.bass_utils import run_bass_kernel_spmd

    nc = _get_graph()
    in_maps = make_in_maps(embeddings, labels, weight)
    res = run_bass_kernel_spmd(nc, in_maps, core_ids=list(range(NCORES)))
    return assemble_output(res.results, embeddings, labels, weight)


if __name__ == "__main__":
    nc = build_graph()
    print("graph built ok")
